# revision 22
# baseline (speedup 1.0000x reference)
"""Trainium2 Bass kernel for nn_LAINRDecoder (sparse attention INR decoder).

Strategy (v2)
-------------
The reference's top-128 sparse attention set is a CONTIGUOUS token window
[s, s+128) with s = clip(floor((idx+1)/4) - 64, 0, 896)  (convex quadratic
bias; verified against jax.lax.top_k in test.py).  Sparse gather-attention
therefore equals dense attention over all 1024 tokens with a per-query
window mask, making every step a dense matmul.

v2 performance structure:
  * all matmuls use float32r (1 cycle/row when output free dim >= 256,
    i.e. 4x faster than plain fp32 at the same 4-byte precision)
  * every matmul has free dim 512 (all queries of a core at once) --
    ~160 matmuls instead of ~1030
  * host-side data layout prep (free): tokens pre-transposed, all weights
    packed into ONE dram tensor (one big DMA that the 16 DMA engines
    split), per-core query grid pre-transposed + window starts
  * the window mask is built directly in (token, query) layout with an
    |x| <= h range test (1 scalar Abs + 1 vector compare per token chunk)
    -- no PE transposes anywhere in the kernel
  * softmax denominators: AV matmul is augmented with a ones column per
    head; reciprocal via exp(-ln(x)) on the scalar engine (the DVE
    reciprocal is ~10 cycles/element)

Sharding: queries split across 8 cores (512 each); both batches processed
by every core; params + tokens replicated.
"""

import math
import os
import sys
import types
from contextlib import ExitStack

import numpy as np

# ---------------------------------------------------------------------------
# environment shims (axon NTFF hook + artifact upload are absent in this
# container; inject them so run_bass_kernel_spmd works with trace=True)
# ---------------------------------------------------------------------------
def _install_shims():
    if "antenv.axon_hooks" not in sys.modules:
        hooks = types.ModuleType("antenv.axon_hooks")
        try:
            from trn_agent_boot.trn_boot import _ntff_profile_via_ctypes

            _hook = _ntff_profile_via_ctypes("/opt/axon/libaxon_pjrt.so")
        except Exception:
            _hook = None
        hooks.get_axon_ntff_profile_hook = lambda: _hook
        hooks.set_axon_ntff_profile_hook = lambda h: None
        sys.modules["antenv.axon_hooks"] = hooks
    import concourse.bass_utils as bass_utils

    bass_utils.upload_artifacts = lambda tmpdir: tmpdir


_install_shims()

import concourse.bass as bass
import concourse.mybir as mybir
import concourse.tile as tile
from concourse.bass_utils import run_bass_kernel_spmd

F32 = mybir.dt.float32
F32R = mybir.dt.float32r
I32 = mybir.dt.int32
AF = mybir.ActivationFunctionType
OP = mybir.AluOpType

# problem constants (hardcoded per the harness contract)
B = 2
Q = 4096
L = 1024
HD = 256
FD = 64
INNER = 128
HEADS = 2
DH = 64
TOPK = 128
N_FREQ = 8
LAYER_NUM = 2
N_CORES = 8
QS = Q // N_CORES          # queries per core (512)
NTOK = L // 128            # token tiles (8)
SCALE = DH ** -0.5

TWO_PI = 2.0 * math.pi
# Cody-Waite split of 2*pi: hi has <=13 mantissa bits so k*hi (k<=64) is exact
TWO_PI_HI = float(np.float32(np.ldexp(np.round(np.ldexp(TWO_PI, 11)), -11)))
TWO_PI_LO = float(np.float32(TWO_PI - TWO_PI_HI))


def _omegas(sigma):
    return np.logspace(1.0, np.log10(sigma), N_FREQ).astype(np.float32)


def _w2(sigma):
    """(4, 64) matrix: arg[q, c*16+j] (j<8 sin slot, j>=8 cos slot) = pi*omega_j*grid[q,c]."""
    w = np.zeros((4, 64), np.float32)
    om = _omegas(sigma)
    for c in range(4):
        for j in range(N_FREQ):
            w[c, c * 16 + j] = np.float32(math.pi) * om[j]
            w[c, c * 16 + 8 + j] = np.float32(math.pi) * om[j]
    return w


def _sincos_bias():
    """(64,1) activation bias: 0 for sin rows, pi/2 for cos rows."""
    b = np.zeros((64, 1), np.float32)
    for c in range(4):
        b[c * 16 + 8 : c * 16 + 16, 0] = np.float32(math.pi / 2)
    return b


# ---------------------------------------------------------------------------
# weight-pack layout: (name -> (col0, ncols)); all tiles live in one
# (128, W_TOTAL) f32 dram tensor / SBUF tile.  Partition layout per entry
# is documented at the pack site.
# ---------------------------------------------------------------------------
def _build_layout():
    lay = {}
    c = 0

    def add(name, ncols):
        nonlocal c
        lay[name] = (c, ncols)
        c += ncols

    add("kvW0", 256)       # (128,256) kv_W[0:128,:]
    add("kvW1", 256)       # (128,256) kv_W[128:256,:]
    add("qW0", 128)        # (128,128) q_W[0:128,:]
    add("qW1", 128)        # (128,128) q_W[128:256,:]
    add("modW", 1024)      # 4 x (128,256): [(l*2+k)*256] = mod_W[l, k*128:(k+1)*128, :]
    add("hvW", 512)        # 2 x (128,256): [k*256] = hv_W[0, k*128:(k+1)*128, :]
    add("olW", 4)          # (128,4): col 2*k+l = outl_W[l, k*128:(k+1)*128, 0]
    add("queryW", 256)     # (64,256) query_W  (partitions 0:64)
    add("outWh", 512)      # 2 x (64,256): [h*256] = out_W[h*64:(h+1)*64,:]
    add("bandW", 512)      # 2 x (64,256): [l*256] = band_W[l]
    add("w2q", 64)         # (4,64)
    add("w2b", 64)         # (4,64)
    add("qb", 2)           # (128,2) query_b chunks
    add("outb", 2)         # (128,2) out_b chunks
    add("bandb", 4)        # (128,4) col l*2+i = band_b[l, i*128:(i+1)*128]
    add("modb", 4)         # (128,4) col l*2+i
    add("hvb", 2)          # (128,2)
    add("scb", 1)          # (64,1) sin/cos phase bias
    add("olb", 1)          # (1,1) sum(outl_b)
    add("ones", 128)       # (1,128) ones row
    add("mbias", 8)        # (128,8): col c = c*128 - 63.5 (mask Abs bias)
    add("onescol", 1)      # (128,1) ones column
    return lay, c


W_LAYOUT, W_TOTAL = _build_layout()


def _pack_weights(inp):
    W = np.zeros((128, W_TOTAL), np.float32)

    def put(name, rows, arr):
        c0, nc_ = W_LAYOUT[name]
        arr = np.asarray(arr, np.float32)
        W[: rows, c0 : c0 + arr.shape[-1]] = arr
        assert arr.shape[-1] <= nc_

    kv_W = np.asarray(inp["kv_W"], np.float32)
    put("kvW0", 128, kv_W[0:128, :])
    put("kvW1", 128, kv_W[128:256, :])
    q_W = np.asarray(inp["q_W"], np.float32)
    put("qW0", 128, q_W[0:128, :])
    put("qW1", 128, q_W[128:256, :])
    mod_W = np.asarray(inp["mod_W"], np.float32)
    mw = np.concatenate([mod_W[l, k * 128 : (k + 1) * 128, :]
                         for l in range(2) for k in range(2)], axis=1)
    put("modW", 128, mw)
    hv_W = np.asarray(inp["hv_W"], np.float32)
    put("hvW", 128, np.concatenate([hv_W[0, k * 128 : (k + 1) * 128, :]
                                    for k in range(2)], axis=1))
    outl_W = np.asarray(inp["outl_W"], np.float32)
    ol = np.zeros((128, 4), np.float32)
    for k in range(2):
        for l in range(2):
            ol[:, 2 * k + l] = outl_W[l, k * 128 : (k + 1) * 128, 0]
    put("olW", 128, ol)
    put("queryW", 64, np.asarray(inp["query_W"], np.float32))
    out_W = np.asarray(inp["out_W"], np.float32)
    put("outWh", 64, np.concatenate([out_W[h * 64 : (h + 1) * 64, :]
                                     for h in range(2)], axis=1))
    band_W = np.asarray(inp["band_W"], np.float32)
    put("bandW", 64, np.concatenate([band_W[0], band_W[1]], axis=1))
    put("w2q", 4, _w2(128.0))
    put("w2b", 4, _w2(32.0))
    qb = np.asarray(inp["query_b"], np.float32)
    put("qb", 128, qb.reshape(2, 128).T)
    ob = np.asarray(inp["out_b"], np.float32)
    put("outb", 128, ob.reshape(2, 128).T)
    bb = np.asarray(inp["band_b"], np.float32)       # (2, 256)
    bbp = np.zeros((128, 4), np.float32)
    for l in range(2):
        for i in range(2):
            bbp[:, l * 2 + i] = bb[l, i * 128 : (i + 1) * 128]
    put("bandb", 128, bbp)
    mb = np.asarray(inp["mod_b"], np.float32)
    mbp = np.zeros((128, 4), np.float32)
    for l in range(2):
        for i in range(2):
            mbp[:, l * 2 + i] = mb[l, i * 128 : (i + 1) * 128]
    put("modb", 128, mbp)
    hb = np.asarray(inp["hv_b"], np.float32)         # (1, 256)
    put("hvb", 128, hb.reshape(2, 128).T)
    put("scb", 64, _sincos_bias())
    put("olb", 1, np.asarray([[np.asarray(inp["outl_b"], np.float32).sum()]]))
    put("ones", 1, np.ones((1, 128), np.float32))
    put("mbias", 128, np.broadcast_to(
        np.arange(8, dtype=np.float32) * 128.0 - 63.5, (128, 8)))
    put("onescol", 128, np.ones((128, 1), np.float32))
    return W


def _window_starts(x0):
    """s = clip((idx+1)//4 - 64, 0, 896) per query; pure integer index math."""
    g = np.asarray(x0, np.float64)
    z = np.floor(g[:, 0] * 8).astype(np.int64)
    y = np.floor(g[:, 1] * 8).astype(np.int64)
    x = np.floor(g[:, 2] * 8).astype(np.int64)
    t = np.floor(g[:, 3] * 8).astype(np.int64)
    idx = ((t * 8 + z) * 8 + y) * 8 + x
    return np.clip((idx + 1) // 4 - 64, 0, 896)


def build_program():
    nc = bass.Bass("TRN2", target_bir_lowering=False, debug=False)

    wpack = nc.dram_tensor("wpack", (128, W_TOTAL), F32R, kind="ExternalInput").ap()
    tokpack = nc.dram_tensor("tokpack", (128, 4 * L), F32R, kind="ExternalInput").ap()
    qgrid = nc.dram_tensor("qgrid", (4, QS), F32R, kind="ExternalInput").ap()
    qsa = nc.dram_tensor("qsa", (1, QS), F32R, kind="ExternalInput").ap()
    out_d = nc.dram_tensor("out", (B, QS), F32, kind="ExternalOutput").ap()

    def wsl(tile_, name, p0=0, np_=128, sub=None):
        """Slice of the weight-pack tile for layout entry `name`."""
        c0, ncols = W_LAYOUT[name]
        if sub is not None:
            c0, ncols = c0 + sub[0], sub[1]
        return tile_[p0 : p0 + np_, c0 : c0 + ncols]

    ctx = ExitStack()
    with tile.TileContext(nc) as tc:
        cpool = ctx.enter_context(tc.tile_pool(name="consts", bufs=1))
        featp = ctx.enter_context(tc.tile_pool(name="feat", bufs=1))
        kvp = ctx.enter_context(tc.tile_pool(name="kv", bufs=1))
        maskp = ctx.enter_context(tc.tile_pool(name="mask", bufs=1))
        ep = ctx.enter_context(tc.tile_pool(name="ep", bufs=3))
        miscp = ctx.enter_context(tc.tile_pool(name="misc", bufs=2))
        pp = ctx.enter_context(tc.tile_pool(name="pp", bufs=3))
        onp = ctx.enter_context(tc.tile_pool(name="on", bufs=1))
        mlp = ctx.enter_context(tc.tile_pool(name="mlt", bufs=2))
        p_big = ctx.enter_context(tc.tile_pool(name="pbig", bufs=4, space="PSUM"))
        p_av = ctx.enter_context(tc.tile_pool(name="pav", bufs=2, space="PSUM"))
        p_row = ctx.enter_context(tc.tile_pool(name="prow", bufs=1, space="PSUM"))

        # ---- input DMAs (one per pack; HW splits across 16 DMA engines) --
        wt = cpool.tile([128, W_TOTAL], F32R, tag="wt", name="wt")
        nc.sync.dma_start(wt[:], wpack[:])
        qg = cpool.tile([4, QS], F32R, tag="qg", name="qg")
        nc.sync.dma_start(qg[:], qgrid[:])
        qs = cpool.tile([1, QS], F32R, tag="qs", name="qs")
        nc.sync.dma_start(qs[:], qsa[:])
        tokt = cpool.tile([128, 4 * L], F32R, tag="tokt", name="tokt")
        nc.sync.dma_start(tokt[:], tokpack[:])

        def r(ap):
            return ap

        def rf(ap):
            return ap.bitcast(F32)

        # ---- constants (no input dependency) ----------------------------
        # iotaP[p, :] = p  (values <= 127, exact in f32)
        iotaP = cpool.tile([128, QS], F32, tag="iotap", name="iotap")
        nc.gpsimd.iota(iotaP[:], pattern=[[0, QS]], base=0, channel_multiplier=1,
                       allow_small_or_imprecise_dtypes=True)

        # V tiles (tok 128, [Vh0 | 1 | Vh1 | 1]); ones columns set up front
        t_V = [[kvp.tile([128, 130], F32R, tag=f"V{b}{c}", name=f"V{b}{c}")
                for c in range(NTOK)] for b in range(B)]
        for b in range(B):
            for c in range(NTOK):
                nc.any.tensor_copy(t_V[b][c][:, 64:65], rf(wsl(wt, "onescol")))
                nc.any.tensor_copy(t_V[b][c][:, 129:130], rf(wsl(wt, "onescol")))

        # ---- window masks in (token, query) layout ----------------------
        # D0[p, q] = p - sA[q]; chunk c in-window <=> |D0 + 128c - 63.5| <= 63.5
        psA = p_big.tile([128, QS], F32, tag="st", name="psA")
        nc.tensor.matmul(psA[:], r(wsl(wt, "ones", 0, 1)), r(qs[:]),
                         start=True, stop=True)
        D0 = maskp.tile([128, QS], F32, tag="D0", name="D0")
        nc.vector.tensor_tensor(D0[:], iotaP[:], psA[:], OP.subtract)
        t_ind = []
        for c in range(NTOK):
            ac = miscp.tile([128, QS], F32, tag="ac", name="ac")
            nc.scalar.activation(ac[:], D0[:], AF.Abs,
                                 bias=rf(wsl(wt, "mbias", 0, 128, (c, 1))))
            ind = maskp.tile([128, QS], F32, tag=f"ind{c}", name=f"ind{c}")
            nc.vector.tensor_scalar(ind[:], ac[:], 63.9, None, OP.is_le)
            t_ind.append(ind)

        # ---- query features (dep: qgrid + wpack only) -------------------
        gridT = qg[:]

        def gamma_T(w2name, tag):
            """(64, 512) = sin(pi*omega*grid + {0,pi/2}) with range reduction."""
            pa = p_big.tile([128, QS], F32, tag="st", name=f"pa_{tag}")
            nc.tensor.matmul(pa[:64, :], r(wsl(wt, w2name, 0, 4)), r(gridT),
                             start=True, stop=True)
            a1 = featp.tile([64, QS], F32, tag=f"{tag}_a1", name=f"{tag}_a1")
            nc.vector.tensor_scalar(a1[:], pa[:64, :], rf(wsl(wt, "scb", 0, 64)),
                                    None, OP.add)
            u = featp.tile([64, QS], F32, tag=f"{tag}_u", name=f"{tag}_u")
            nc.vector.tensor_scalar(u[:], a1[:], 1.0 / TWO_PI, None, OP.mult)
            ki = featp.tile([64, QS], I32, tag=f"{tag}_ki", name=f"{tag}_ki")
            nc.vector.tensor_copy(ki[:], u[:])  # round-to-nearest
            kf = featp.tile([64, QS], F32, tag=f"{tag}_kf", name=f"{tag}_kf")
            nc.vector.tensor_copy(kf[:], ki[:])
            nc.vector.tensor_scalar(u[:], kf[:], TWO_PI_HI, None, OP.mult)
            nc.vector.tensor_tensor(a1[:], a1[:], u[:], OP.subtract)
            nc.vector.tensor_scalar(u[:], kf[:], TWO_PI_LO, None, OP.mult)
            nc.vector.tensor_tensor(a1[:], a1[:], u[:], OP.subtract)
            g = featp.tile([64, QS], F32R, tag=f"{tag}_g", name=f"{tag}_g")
            nc.scalar.activation(g[:], a1[:], AF.Sin)
            return g

        gq = gamma_T("w2q", "gq")      # used by attention AND band layer 0
        gb1 = gamma_T("w2b", "gb1")    # band layer 1

        # x_qT (256, 512) = relu(query_W^T @ gammaT + qb)
        x_qT = [featp.tile([128, QS], F32R, tag=f"xq{i}", name=f"xq{i}") for i in range(2)]
        for i in range(2):
            px = p_big.tile([128, QS], F32, tag="st", name=f"px{i}")
            nc.tensor.matmul(px[:], r(wsl(wt, "queryW", 0, 64, (i * 128, 128))),
                             r(gq[:]), start=True, stop=True)
            nc.scalar.activation(x_qT[i][:], px[:], AF.Relu,
                                 bias=rf(wsl(wt, "qb", 0, 128, (i, 1))))
        # qT (128, 512) = (q_W^T @ x_qT) * SCALE
        qT = featp.tile([INNER, QS], F32R, tag="qT", name="qT")
        pq = p_big.tile([128, QS], F32, tag="st", name="pq")
        for k in range(2):
            nc.tensor.matmul(pq[:], r(wsl(wt, f"qW{k}")), r(x_qT[k][:]),
                             start=(k == 0), stop=(k == 1))
        nc.scalar.activation(qT[:], pq[:], AF.Copy, scale=SCALE)
        # band features h_lT (2 layers x 2 chunks of (128, 512))
        h_lT = [[featp.tile([128, QS], F32, tag=f"hl{l}{i}", name=f"hl{l}{i}")
                 for i in range(2)] for l in range(2)]
        for l, gsrc in ((0, gq), (1, gb1)):
            for i in range(2):
                ph = p_big.tile([128, QS], F32, tag="st", name=f"ph{l}{i}")
                nc.tensor.matmul(ph[:], r(wsl(wt, "bandW", 0, 64,
                                              (l * 256 + i * 128, 128))),
                                 r(gsrc[:]), start=True, stop=True)
                nc.scalar.activation(h_lT[l][i][:], ph[:], AF.Relu,
                                     bias=rf(wsl(wt, "bandb", 0, 128, (l * 2 + i, 1))))

        # ---- KV setup (dep: tokpack) ------------------------------------
        # tokt cols (b*2+k)*1024 + t hold tokens[b, t, k*128+p]
        t_KT = [kvp.tile([128, L], F32R, tag=f"KT{b}", name=f"KT{b}") for b in range(B)]
        for b in range(B):
            for half in range(2):
                pk = p_big.tile([128, QS], F32, tag="st", name=f"pk{b}{half}")
                for k in range(2):
                    nc.tensor.matmul(
                        pk[:], r(wsl(wt, f"kvW{k}", 0, 128, (0, 128))),
                        r(tokt[:, (b * 2 + k) * L + half * 512 :
                               (b * 2 + k) * L + half * 512 + 512]),
                        start=(k == 0), stop=(k == 1))
                nc.scalar.copy(t_KT[b][:, half * 512 : half * 512 + 512], pk[:])
            for c in range(NTOK):
                pvt = p_big.tile([128, QS], F32, tag="st", name=f"pv{b}{c}")
                pv = pvt[:, 0:128]
                for k in range(2):
                    nc.tensor.matmul(
                        pv, r(tokt[:, (b * 2 + k) * L + c * 128 :
                                   (b * 2 + k) * L + c * 128 + 128]),
                        r(wsl(wt, f"kvW{k}", 0, 128, (128, 128))),
                        start=(k == 0), stop=(k == 1))
                nc.any.tensor_copy(t_V[b][c][:, 0:64], pvt[:, 0:64])
                nc.any.tensor_copy(t_V[b][c][:, 65:129], pvt[:, 64:128])

        # ---- attention per (batch, head) --------------------------------
        # softmax denominator reciprocal via exp(-ln(x)) on the scalar
        # engine; normalize right away so the AV psum frees quickly.
        oN = {}
        for b in range(B):
            for h in range(HEADS):
                pot = p_av.tile([65, QS], F32, tag="ot", name=f"ot{b}{h}")
                for c in range(NTOK):
                    st = p_big.tile([128, QS], F32, tag="st", name=f"st{b}{h}{c}")
                    nc.tensor.matmul(
                        st[:],
                        r(t_KT[b][h * 64 : (h + 1) * 64, c * 128 : (c + 1) * 128]),
                        r(qT[h * 64 : (h + 1) * 64, :]),
                        start=True, stop=True)
                    e = ep.tile([128, QS], F32, tag="e", name="e")
                    nc.scalar.activation(e[:], st[:], AF.Exp)
                    P = pp.tile([128, QS], F32R, tag="P", name="P")
                    nc.vector.tensor_tensor(P[:], e[:], t_ind[c][:], OP.mult)
                    nc.tensor.matmul(
                        pot[:], r(t_V[b][c][:, h * 65 : (h + 1) * 65]), r(P[:]),
                        start=(c == 0), stop=(c == NTOK - 1))
                dr = miscp.tile([1, QS], F32, tag="dr", name="dr")
                nc.any.tensor_copy(dr[:], pot[64:65, :])
                nc.scalar.activation(dr[:], dr[:], AF.Ln)
                inv = miscp.tile([1, QS], F32R, tag="inv", name="inv")
                nc.scalar.activation(inv[:], dr[:], AF.Exp, scale=-1.0)
                pbc = p_big.tile([64, QS], F32, tag="st", name=f"pbc{b}{h}")
                nc.tensor.matmul(pbc[:], r(wsl(wt, "ones", 0, 1, (0, 64))),
                                 r(inv[:]), start=True, stop=True)
                bcs = miscp.tile([64, QS], F32, tag="bcs", name="bcs")
                nc.scalar.copy(bcs[:], pbc[:])
                onh = onp.tile([64, QS], F32R, tag=f"on{b}{h}", name=f"on{b}{h}")
                nc.vector.tensor_tensor(onh[:], pot[0:64, :], bcs[:],
                                        OP.mult)
                oN[(b, h)] = onh

        # ---- MLP tail per batch (512-wide) ------------------------------
        orows = [mlp.tile([1, QS], F32, tag=f"orow{b}", name=f"orow{b}")
                 for b in range(B)]
        for b in range(B):
            # modT (2 chunks of (128,512)) = out_W^T @ oN + out_b
            modT = []
            for mc in range(2):
                pm = p_big.tile([128, QS], F32, tag="st", name=f"pm{b}{mc}")
                for h in range(2):
                    nc.tensor.matmul(
                        pm[:], r(wsl(wt, "outWh", 0, 64, (h * 256 + mc * 128, 128))),
                        r(oN[(b, h)][:]), start=(h == 0), stop=(h == 1))
                mt = mlp.tile([128, QS], F32R, tag=f"modT{mc}", name=f"modT{b}{mc}")
                nc.scalar.activation(mt[:], pm[:], AF.Identity,
                                     bias=rf(wsl(wt, "outb", 0, 128, (mc, 1))))
                modT.append(mt)
            # m_l = relu(h_l + modT @ mod_W + mod_b)
            mls = [[None, None], [None, None]]
            for l in range(2):
                for mc in range(2):
                    pm = p_big.tile([128, QS], F32, tag="st", name=f"pml{b}{l}{mc}")
                    for k in range(2):
                        nc.tensor.matmul(
                            pm[:], r(wsl(wt, "modW", 0, 128,
                                         ((l * 2 + k) * 256 + mc * 128, 128))),
                            r(modT[k][:]), start=(k == 0), stop=(k == 1))
                    tadd = miscp.tile([128, QS], F32, tag="tadd", name="tadd")
                    nc.vector.tensor_tensor(tadd[:], pm[:], h_lT[l][mc][:], OP.add)
                    ml = mlp.tile([128, QS], F32R, tag=f"ml{l}{mc}", name=f"ml{b}{l}{mc}")
                    nc.scalar.activation(ml[:], tadd[:], AF.Relu,
                                         bias=rf(wsl(wt, "modb", 0, 128, (l * 2 + mc, 1))))
                    mls[l][mc] = ml
            # h_v1 = relu((m0 + m1) @ hv_W + hv_b)
            sum01 = []
            for mc in range(2):
                s01 = miscp.tile([128, QS], F32R, tag="s01", name="s01")
                nc.vector.tensor_tensor(s01[:], rf(mls[0][mc][:]), rf(mls[1][mc][:]), OP.add)
                sum01.append(s01)
            hv1 = []
            for mc in range(2):
                pm = p_big.tile([128, QS], F32, tag="st", name=f"phv{b}{mc}")
                for k in range(2):
                    nc.tensor.matmul(
                        pm[:], r(wsl(wt, "hvW", 0, 128, (k * 256 + mc * 128, 128))),
                        r(sum01[k][:]), start=(k == 0), stop=(k == 1))
                hv = mlp.tile([128, QS], F32R, tag=f"hv{mc}", name=f"hv{b}{mc}")
                nc.scalar.activation(hv[:], pm[:], AF.Relu,
                                     bias=rf(wsl(wt, "hvb", 0, 128, (mc, 1))))
                hv1.append(hv)
            # out row = h_v0 @ outl_W[0] + h_v1 @ outl_W[1] + sum(outl_b)
            por = p_row.tile([1, QS], F32, tag="por", name=f"por{b}")
            steps = [(wsl(wt, "olW", 0, 128, (2 * k, 1)), mls[0][k]) for k in range(2)] + \
                    [(wsl(wt, "olW", 0, 128, (2 * k + 1, 1)), hv1[k]) for k in range(2)]
            for si, (lw, rv) in enumerate(steps):
                nc.tensor.matmul(por[:], r(lw), r(rv[:]), start=(si == 0),
                                 stop=(si == len(steps) - 1))
            nc.scalar.activation(orows[b][:], por[:], AF.Identity,
                                 bias=rf(wsl(wt, "olb", 0, 1)))
        for b in range(B):
            nc.sync.dma_start(out_d[b : b + 1, :], orows[b][:])
        ctx.close()

    _split_multi_waits_inline(nc)
    return nc


def _split_multi_waits_inline(nc):
    """Self-contained copy of the wait-splitting post-pass."""
    for fn in nc.m.functions:
        for blk in fn.blocks:
            new_insts = []
            for inst in blk.instructions:
                si = getattr(inst, "sync_info", None)
                if si is not None and len(si.on_wait) > 1:
                    waits = list(si.on_wait)
                    for j, w in enumerate(waits[:-1]):
                        new_insts.append(mybir.InstNoOp(
                            name=f"{inst.name}-ws{j}",
                            engine=inst.engine,
                            sync_info=mybir.SyncInfo(on_wait=[w], on_update=[]),
                            bass_nofuse=True,
                        ))
                    si.on_wait = waits[-1:]
                new_insts.append(inst)
            blk.instructions = new_insts


_CACHED_NC = None
LAST_RESULTS = None


def kernel(**inputs):
    global _CACHED_NC
    x = np.asarray(inputs["x"], np.float32)
    tokens = np.asarray(inputs["tokens"], np.float32)
    assert int(inputs["gD"]) == 8 and int(inputs["gH"]) == 8
    assert int(inputs["gW"]) == 8 and int(inputs["gT"]) == 8

    if _CACHED_NC is None:
        _CACHED_NC = build_program()
    nc = _CACHED_NC

    x0 = np.ascontiguousarray(x[0])  # (Q, 4) — reference uses x[0] for all batches
    s_all = _window_starts(x0)

    # tokens pre-transposed + packed: col (b*2+k)*1024 + t <- tokens[b, t, k*128+p]
    tt = tokens.transpose(0, 2, 1)   # (B, 256, 1024)
    tokpack = np.ascontiguousarray(
        np.concatenate([tt[0, 0:128], tt[0, 128:256],
                        tt[1, 0:128], tt[1, 128:256]], axis=1))
    wpack = _pack_weights(inputs)

    in_maps = []
    for c in range(N_CORES):
        sl = slice(c * QS, (c + 1) * QS)
        in_maps.append({
            "wpack": wpack,
            "tokpack": tokpack,
            "qgrid": np.ascontiguousarray(x0[sl].T),
            "qsa": np.ascontiguousarray(s_all[sl].astype(np.float32)[None, :]),
        })

    global LAST_RESULTS
    trace = bool(os.environ.get("KERNEL_TRACE"))
    res = run_bass_kernel_spmd(nc, in_maps, core_ids=list(range(N_CORES)),
                               trace=trace)
    LAST_RESULTS = res
    parts = [res.results[c]["out"] for c in range(N_CORES)]  # each (B, QS)
    out = np.concatenate(parts, axis=1).reshape(B, Q, 1).astype(np.float32)
    return out


# revision 23
# speedup vs baseline: 1.2774x; 1.2774x over previous
"""Trainium2 Bass kernel for nn_LAINRDecoder (sparse attention INR decoder).

Strategy (v3)
-------------
The reference's top-128 sparse attention set is a CONTIGUOUS token window
[s, s+128) with s = clip(floor((idx+1)/4) - 64, 0, 896)  (convex quadratic
bias; verified against jax.lax.top_k in test.py).  Sparse gather-attention
therefore equals dense attention with a per-query window mask.

v3 performance structure:
  * queries are SORTED by window start s on the host and sharded in sorted
    order, so each core's 512 queries touch a narrow contiguous token range
    (<= 267 tokens for uniform grids).  Each core receives only its own
    NCH*128-token slice of the tokens (host-sliced, data-parallel SPMD) and
    runs dense attention over NCH=3 chunks instead of 8.  The host
    un-permutes the output columns.  If an input distribution ever needs a
    wider range, the program is rebuilt with a bigger NCH (cached per NCH).
  * all matmuls are float32r (1 cycle/row at >= 256 free dim; 4x fp32) with
    free dim 512 (all queries of a core at once)
  * the window mask is applied as an additive -1e6 before exp (one vector
    add), not a multiply after it -- and is built in transposed (token,
    query) layout with an |x| <= h range test; no PE transposes anywhere
  * softmax denominators via a ones-augmented AV column; reciprocal as
    exp(-ln(x)) on the scalar engine (DVE reciprocal is ~10 cyc/element)
  * host packs all weights into two dram tensors (early / tail) so one DMA
    instruction each fans out across the 16 DMA engines; tokens pre-
    transposed per batch.
"""

import math
import os
import sys
import types
from contextlib import ExitStack

import numpy as np

# ---------------------------------------------------------------------------
# environment shims (axon NTFF hook + artifact upload are absent in this
# container; inject them so run_bass_kernel_spmd works with trace=True)
# ---------------------------------------------------------------------------
def _install_shims():
    if "antenv.axon_hooks" not in sys.modules:
        hooks = types.ModuleType("antenv.axon_hooks")
        try:
            from trn_agent_boot.trn_boot import _ntff_profile_via_ctypes

            _hook = _ntff_profile_via_ctypes("/opt/axon/libaxon_pjrt.so")
        except Exception:
            _hook = None
        hooks.get_axon_ntff_profile_hook = lambda: _hook
        hooks.set_axon_ntff_profile_hook = lambda h: None
        sys.modules["antenv.axon_hooks"] = hooks
    import concourse.bass_utils as bass_utils

    bass_utils.upload_artifacts = lambda tmpdir: tmpdir


_install_shims()

import concourse.bass as bass
import concourse.mybir as mybir
import concourse.tile as tile
from concourse.bass_utils import run_bass_kernel_spmd

F32 = mybir.dt.float32
F32R = mybir.dt.float32r
AF = mybir.ActivationFunctionType
OP = mybir.AluOpType

# problem constants (hardcoded per the harness contract)
B = 2
Q = 4096
L = 1024
HD = 256
FD = 64
INNER = 128
HEADS = 2
DH = 64
TOPK = 128
N_FREQ = 8
LAYER_NUM = 2
N_CORES = 8
QS = Q // N_CORES          # queries per core (512)
SCALE = DH ** -0.5
NEG_BIG = -1.0e6           # additive mask for out-of-window logits

TWO_PI = 2.0 * math.pi
# Cody-Waite split of 2*pi: hi has <=13 mantissa bits so k*hi (k<=64) is exact
TWO_PI_HI = float(np.float32(np.ldexp(np.round(np.ldexp(TWO_PI, 11)), -11)))
TWO_PI_LO = float(np.float32(TWO_PI - TWO_PI_HI))


def _omegas(sigma):
    return np.logspace(1.0, np.log10(sigma), N_FREQ).astype(np.float32)


def _w2(sigma):
    """(4, 64) matrix: arg[q, c*16+j] (j<8 sin slot, j>=8 cos slot) = pi*omega_j*grid[q,c]."""
    w = np.zeros((4, 64), np.float32)
    om = _omegas(sigma)
    for c in range(4):
        for j in range(N_FREQ):
            w[c, c * 16 + j] = np.float32(math.pi) * om[j]
            w[c, c * 16 + 8 + j] = np.float32(math.pi) * om[j]
    return w


def _sincos_bias():
    """(64,1) activation bias: 0 for sin rows, pi/2 for cos rows."""
    b = np.zeros((64, 1), np.float32)
    for c in range(4):
        b[c * 16 + 8 : c * 16 + 16, 0] = np.float32(math.pi / 2)
    return b


# ---------------------------------------------------------------------------
# weight-pack layout: name -> (group, col0, ncols).  Group 0 = needed early
# (attention path), group 1 = tail-only.  One (128, Wg_TOTAL) f32 dram
# tensor + DMA per group.
# ---------------------------------------------------------------------------
def _build_layout():
    lay = {}
    cols = [0, 0]

    def add(g, name, ncols):
        lay[name] = (g, cols[g], ncols)
        cols[g] += ncols

    # group 0: attention path
    add(0, "kvW0", 256)       # (128,256) kv_W[0:128,:]
    add(0, "kvW1", 256)       # (128,256) kv_W[128:256,:]
    add(0, "qW0", 128)        # (128,128) q_W[0:128,:]
    add(0, "qW1", 128)        # (128,128) q_W[128:256,:]
    add(0, "queryW", 256)     # (64,256) query_W  (partitions 0:64)
    add(0, "w2q", 64)         # (4,64)
    add(0, "w2b", 64)         # (4,64)
    add(0, "qb", 2)           # (128,2) query_b chunks
    add(0, "scb", 1)          # (64,1) sin/cos phase bias
    add(0, "ones", 128)       # (1,128) ones row
    add(0, "mbias", 8)        # (128,8): col c = c*128 - 63.5 (mask Abs bias)
    add(0, "onescol", 1)      # (128,1) ones column
    # group 1: MLP tail
    add(1, "modW", 1024)      # 4 x (128,256): [(l*2+k)*256] = mod_W[l, k*128:(k+1)*128, :]
    add(1, "hvW", 512)        # 2 x (128,256): [k*256] = hv_W[0, k*128:(k+1)*128, :]
    add(1, "olW", 4)          # (128,4): col 2*k+l = outl_W[l, k*128:(k+1)*128, 0]
    add(1, "outWh", 512)      # 2 x (64,256): [h*256] = out_W[h*64:(h+1)*64,:]
    add(1, "bandW", 512)      # 2 x (64,256): [l*256] = band_W[l]
    add(1, "outb", 2)         # (128,2) out_b chunks
    add(1, "bandb", 4)        # (128,4) col l*2+i = band_b[l, i*128:(i+1)*128]
    add(1, "modb", 4)         # (128,4) col l*2+i
    add(1, "hvb", 2)          # (128,2)
    add(1, "olb", 1)          # (1,1) sum(outl_b)
    return lay, cols


W_LAYOUT, W_TOTALS = _build_layout()


def _pack_weights(inp):
    W = [np.zeros((128, W_TOTALS[0]), np.float32),
         np.zeros((128, W_TOTALS[1]), np.float32)]

    def put(name, arr):
        g, c0, nc_ = W_LAYOUT[name]
        arr = np.asarray(arr, np.float32)
        assert arr.shape[-1] <= nc_
        W[g][: arr.shape[0], c0 : c0 + arr.shape[-1]] = arr

    kv_W = np.asarray(inp["kv_W"], np.float32)
    put("kvW0", kv_W[0:128, :])
    put("kvW1", kv_W[128:256, :])
    q_W = np.asarray(inp["q_W"], np.float32)
    put("qW0", q_W[0:128, :])
    put("qW1", q_W[128:256, :])
    put("queryW", np.asarray(inp["query_W"], np.float32))
    put("w2q", _w2(128.0))
    put("w2b", _w2(32.0))
    put("qb", np.asarray(inp["query_b"], np.float32).reshape(2, 128).T)
    put("scb", _sincos_bias())
    put("ones", np.ones((1, 128), np.float32))
    put("mbias", np.broadcast_to(
        np.arange(8, dtype=np.float32) * 128.0 - 63.5, (128, 8)))
    put("onescol", np.ones((128, 1), np.float32))

    mod_W = np.asarray(inp["mod_W"], np.float32)
    put("modW", np.concatenate([mod_W[l, k * 128 : (k + 1) * 128, :]
                                for l in range(2) for k in range(2)], axis=1))
    hv_W = np.asarray(inp["hv_W"], np.float32)
    put("hvW", np.concatenate([hv_W[0, k * 128 : (k + 1) * 128, :]
                               for k in range(2)], axis=1))
    outl_W = np.asarray(inp["outl_W"], np.float32)
    ol = np.zeros((128, 4), np.float32)
    for k in range(2):
        for l in range(2):
            ol[:, 2 * k + l] = outl_W[l, k * 128 : (k + 1) * 128, 0]
    put("olW", ol)
    out_W = np.asarray(inp["out_W"], np.float32)
    put("outWh", np.concatenate([out_W[h * 64 : (h + 1) * 64, :]
                                 for h in range(2)], axis=1))
    band_W = np.asarray(inp["band_W"], np.float32)
    put("bandW", np.concatenate([band_W[0], band_W[1]], axis=1))
    put("outb", np.asarray(inp["out_b"], np.float32).reshape(2, 128).T)
    bb = np.asarray(inp["band_b"], np.float32)
    put("bandb", np.stack([bb[l, i * 128 : (i + 1) * 128]
                           for l in range(2) for i in range(2)], axis=1))
    mb = np.asarray(inp["mod_b"], np.float32)
    put("modb", np.stack([mb[l, i * 128 : (i + 1) * 128]
                          for l in range(2) for i in range(2)], axis=1))
    put("hvb", np.asarray(inp["hv_b"], np.float32).reshape(2, 128).T)
    put("olb", np.asarray([[np.asarray(inp["outl_b"], np.float32).sum()]]))
    return W


def _window_starts(x0):
    """s = clip((idx+1)//4 - 64, 0, 896) per query; pure integer index math."""
    g = np.asarray(x0, np.float64)
    z = np.floor(g[:, 0] * 8).astype(np.int64)
    y = np.floor(g[:, 1] * 8).astype(np.int64)
    x = np.floor(g[:, 2] * 8).astype(np.int64)
    t = np.floor(g[:, 3] * 8).astype(np.int64)
    idx = ((t * 8 + z) * 8 + y) * 8 + x
    return np.clip((idx + 1) // 4 - 64, 0, 896)


def build_program(nch):
    """nch = number of 128-token chunks each core's attention covers."""
    LT = nch * 128            # tokens per core slice
    nc = bass.Bass("TRN2", target_bir_lowering=False, debug=False)

    wp = [nc.dram_tensor(f"wpack{g}", (128, W_TOTALS[g]), F32R,
                         kind="ExternalInput").ap() for g in range(2)]
    tokp = [nc.dram_tensor(f"tokpack{b}", (128, 2 * LT), F32R,
                           kind="ExternalInput").ap() for b in range(B)]
    qgrid = nc.dram_tensor("qgrid", (4, QS), F32R, kind="ExternalInput").ap()
    qsa = nc.dram_tensor("qsa", (1, QS), F32R, kind="ExternalInput").ap()
    out_d = nc.dram_tensor("out", (B, QS), F32, kind="ExternalOutput").ap()

    ctx = ExitStack()
    with tile.TileContext(nc) as tc:
        cpool = ctx.enter_context(tc.tile_pool(name="consts", bufs=1))
        featp = ctx.enter_context(tc.tile_pool(name="feat", bufs=1))
        kvp = ctx.enter_context(tc.tile_pool(name="kv", bufs=1))
        maskp = ctx.enter_context(tc.tile_pool(name="mask", bufs=1))
        ep = ctx.enter_context(tc.tile_pool(name="ep", bufs=3))
        miscp = ctx.enter_context(tc.tile_pool(name="misc", bufs=2))
        pp = ctx.enter_context(tc.tile_pool(name="pp", bufs=3))
        onp = ctx.enter_context(tc.tile_pool(name="on", bufs=1))
        mlp = ctx.enter_context(tc.tile_pool(name="mlt", bufs=2))
        p_big = ctx.enter_context(tc.tile_pool(name="pbig", bufs=4, space="PSUM"))
        p_av = ctx.enter_context(tc.tile_pool(name="pav", bufs=2, space="PSUM"))
        p_row = ctx.enter_context(tc.tile_pool(name="prow", bufs=1, space="PSUM"))

        # ---- input DMAs (per-pack; HW fans each across 16 DMA engines) --
        qg = cpool.tile([4, QS], F32R, tag="qg", name="qg")
        nc.sync.dma_start(qg[:], qgrid[:])
        qs = cpool.tile([1, QS], F32R, tag="qs", name="qs")
        nc.sync.dma_start(qs[:], qsa[:])
        wt0 = cpool.tile([128, W_TOTALS[0]], F32R, tag="wt0", name="wt0")
        nc.sync.dma_start(wt0[:], wp[0][:])
        tokt = [cpool.tile([128, 2 * LT], F32R, tag=f"tokt{b}", name=f"tokt{b}")
                for b in range(B)]
        for b in range(B):
            nc.sync.dma_start(tokt[b][:], tokp[b][:])
        wt1 = cpool.tile([128, W_TOTALS[1]], F32R, tag="wt1", name="wt1")
        nc.sync.dma_start(wt1[:], wp[1][:])

        def wsl(name, p0=0, np_=128, sub=None):
            g, c0, ncols = W_LAYOUT[name]
            t_ = wt0 if g == 0 else wt1
            if sub is not None:
                c0, ncols = c0 + sub[0], sub[1]
            return t_[p0 : p0 + np_, c0 : c0 + ncols]

        def rf(ap):
            return ap.bitcast(F32)

        # ---- constants (no input dependency) ----------------------------
        # iotaP[p, :] = p  (values <= 127, exact in f32)
        iotaP = cpool.tile([128, QS], F32, tag="iotap", name="iotap")
        nc.gpsimd.iota(iotaP[:], pattern=[[0, QS]], base=0, channel_multiplier=1,
                       allow_small_or_imprecise_dtypes=True)

        # V tiles (tok 128, [Vh0 | 1 | Vh1 | 1]); ones columns set up front
        t_V = [[kvp.tile([128, 130], F32R, tag=f"V{b}{c}", name=f"V{b}{c}")
                for c in range(nch)] for b in range(B)]
        for b in range(B):
            for c in range(nch):
                nc.any.tensor_copy(t_V[b][c][:, 64:65], rf(wsl("onescol")))
                nc.any.tensor_copy(t_V[b][c][:, 129:130], rf(wsl("onescol")))

        # ---- additive window masks in (token, query) layout -------------
        # D0[p, q] = p - sA[q]; chunk c out-of-window <=> |D0 + 128c - 63.5| > 63.5
        # lni = 0 in-window, NEG_BIG outside (added to logits before exp)
        psA = p_big.tile([128, QS], F32, tag="st", name="psA")
        nc.tensor.matmul(psA[:], wsl("ones", 0, 1), qs[:], start=True, stop=True)
        D0 = maskp.tile([128, QS], F32, tag="D0", name="D0")
        nc.vector.tensor_tensor(D0[:], iotaP[:], psA[:], OP.subtract)
        t_lni = []
        for c in range(nch):
            ac = miscp.tile([128, QS], F32, tag="ac", name="ac")
            nc.scalar.activation(ac[:], D0[:], AF.Abs,
                                 bias=rf(wsl("mbias", 0, 128, (c, 1))))
            lni = maskp.tile([128, QS], F32, tag=f"lni{c}", name=f"lni{c}")
            nc.vector.tensor_scalar(lni[:], ac[:], 63.9, None, OP.is_gt)
            nc.vector.tensor_scalar(lni[:], lni[:], NEG_BIG, None, OP.mult)
            t_lni.append(lni)

        # ---- query features (dep: qgrid + wpack0) -----------------------
        def gamma_T(w2name, tag):
            """(64, 512) = sin(pi*omega*grid + {0,pi/2}) with range reduction."""
            pa = p_big.tile([128, QS], F32, tag="st", name=f"pa_{tag}")
            nc.tensor.matmul(pa[:64, :], wsl(w2name, 0, 4), qg[:],
                             start=True, stop=True)
            a1 = featp.tile([64, QS], F32, tag=f"{tag}_a1", name=f"{tag}_a1")
            nc.vector.tensor_scalar(a1[:], pa[:64, :], rf(wsl("scb", 0, 64)),
                                    None, OP.add)
            u = featp.tile([64, QS], F32, tag=f"{tag}_u", name=f"{tag}_u")
            nc.vector.tensor_scalar(u[:], a1[:], 1.0 / TWO_PI, None, OP.mult)
            ki = featp.tile([64, QS], mybir.dt.int32, tag=f"{tag}_ki", name=f"{tag}_ki")
            nc.vector.tensor_copy(ki[:], u[:])  # round-to-nearest
            kf = featp.tile([64, QS], F32, tag=f"{tag}_kf", name=f"{tag}_kf")
            nc.vector.tensor_copy(kf[:], ki[:])
            nc.vector.tensor_scalar(u[:], kf[:], TWO_PI_HI, None, OP.mult)
            nc.vector.tensor_tensor(a1[:], a1[:], u[:], OP.subtract)
            nc.vector.tensor_scalar(u[:], kf[:], TWO_PI_LO, None, OP.mult)
            nc.vector.tensor_tensor(a1[:], a1[:], u[:], OP.subtract)
            g = featp.tile([64, QS], F32R, tag=f"{tag}_g", name=f"{tag}_g")
            nc.scalar.activation(g[:], a1[:], AF.Sin)
            return g

        gq = gamma_T("w2q", "gq")      # used by attention AND band layer 0
        gb1 = gamma_T("w2b", "gb1")    # band layer 1

        # x_qT (256, 512) = relu(query_W^T @ gammaT + qb)
        x_qT = [featp.tile([128, QS], F32R, tag=f"xq{i}", name=f"xq{i}")
                for i in range(2)]
        for i in range(2):
            px = p_big.tile([128, QS], F32, tag="st", name=f"px{i}")
            nc.tensor.matmul(px[:], wsl("queryW", 0, 64, (i * 128, 128)),
                             gq[:], start=True, stop=True)
            nc.scalar.activation(x_qT[i][:], px[:], AF.Relu,
                                 bias=rf(wsl("qb", 0, 128, (i, 1))))
        # qT (128, 512) = (q_W^T @ x_qT) * SCALE
        qT = featp.tile([INNER, QS], F32R, tag="qT", name="qT")
        pq = p_big.tile([128, QS], F32, tag="st", name="pq")
        for k in range(2):
            nc.tensor.matmul(pq[:], wsl(f"qW{k}"), x_qT[k][:],
                             start=(k == 0), stop=(k == 1))
        nc.scalar.activation(qT[:], pq[:], AF.Copy, scale=SCALE)
        # band features h_lT (2 layers x 2 chunks of (128, 512))
        h_lT = [[featp.tile([128, QS], F32, tag=f"hl{l}{i}", name=f"hl{l}{i}")
                 for i in range(2)] for l in range(2)]
        for l, gsrc in ((0, gq), (1, gb1)):
            for i in range(2):
                ph = p_big.tile([128, QS], F32, tag="st", name=f"ph{l}{i}")
                nc.tensor.matmul(ph[:], wsl("bandW", 0, 64, (l * 256 + i * 128, 128)),
                                 gsrc[:], start=True, stop=True)
                nc.scalar.activation(h_lT[l][i][:], ph[:], AF.Relu,
                                     bias=rf(wsl("bandb", 0, 128, (l * 2 + i, 1))))

        # ---- KV setup (dep: tokpack[b]) ---------------------------------
        # tokt[b] cols k*LT + t hold tokens[b, cbase+t, k*128+p]
        t_KT = [kvp.tile([128, LT], F32R, tag=f"KT{b}", name=f"KT{b}")
                for b in range(B)]
        for b in range(B):
            pk = p_big.tile([128, LT], F32, tag="st", name=f"pk{b}")
            for k in range(2):
                nc.tensor.matmul(pk[:], wsl(f"kvW{k}", 0, 128, (0, 128)),
                                 tokt[b][:, k * LT : (k + 1) * LT],
                                 start=(k == 0), stop=(k == 1))
            nc.scalar.copy(t_KT[b][:], pk[:])
            for c in range(nch):
                pvt = p_big.tile([128, QS], F32, tag="st", name=f"pv{b}{c}")
                pv = pvt[:, 0:128]
                for k in range(2):
                    nc.tensor.matmul(
                        pv, tokt[b][:, k * LT + c * 128 : k * LT + c * 128 + 128],
                        wsl(f"kvW{k}", 0, 128, (128, 128)),
                        start=(k == 0), stop=(k == 1))
                nc.any.tensor_copy(t_V[b][c][:, 0:64], pvt[:, 0:64])
                nc.any.tensor_copy(t_V[b][c][:, 65:129], pvt[:, 64:128])

        # ---- attention per (batch, head) --------------------------------
        oN = {}
        for b in range(B):
            for h in range(HEADS):
                pot = p_av.tile([65, QS], F32, tag="ot", name=f"ot{b}{h}")
                for c in range(nch):
                    st = p_big.tile([128, QS], F32, tag="st", name=f"st{b}{h}{c}")
                    nc.tensor.matmul(
                        st[:],
                        t_KT[b][h * 64 : (h + 1) * 64, c * 128 : (c + 1) * 128],
                        qT[h * 64 : (h + 1) * 64, :],
                        start=True, stop=True)
                    sm = ep.tile([128, QS], F32, tag="sm", name="sm")
                    nc.vector.tensor_tensor(sm[:], st[:], t_lni[c][:], OP.add)
                    P = pp.tile([128, QS], F32R, tag="P", name="P")
                    nc.scalar.activation(P[:], sm[:], AF.Exp)
                    nc.tensor.matmul(
                        pot[:], t_V[b][c][:, h * 65 : (h + 1) * 65], P[:],
                        start=(c == 0), stop=(c == nch - 1))
                # denominator reciprocal via exp(-ln(x)); normalize now so
                # the AV psum frees quickly
                dr = miscp.tile([1, QS], F32, tag="dr", name="dr")
                nc.any.tensor_copy(dr[:], pot[64:65, :])
                nc.scalar.activation(dr[:], dr[:], AF.Ln)
                inv = miscp.tile([1, QS], F32R, tag="inv", name="inv")
                nc.scalar.activation(inv[:], dr[:], AF.Exp, scale=-1.0)
                pbc = p_big.tile([64, QS], F32, tag="st", name=f"pbc{b}{h}")
                nc.tensor.matmul(pbc[:], wsl("ones", 0, 1, (0, 64)), inv[:],
                                 start=True, stop=True)
                bcs = miscp.tile([64, QS], F32, tag="bcs", name="bcs")
                nc.scalar.copy(bcs[:], pbc[:])
                onh = onp.tile([64, QS], F32R, tag=f"on{b}{h}", name=f"on{b}{h}")
                nc.vector.tensor_tensor(onh[:], pot[0:64, :], bcs[:], OP.mult)
                oN[(b, h)] = onh

        # ---- MLP tail per batch (512-wide) ------------------------------
        orows = [mlp.tile([1, QS], F32, tag=f"orow{b}", name=f"orow{b}")
                 for b in range(B)]
        for b in range(B):
            # modT (2 chunks of (128,512)) = out_W^T @ oN + out_b
            modT = []
            for mc in range(2):
                pm = p_big.tile([128, QS], F32, tag="st", name=f"pm{b}{mc}")
                for h in range(2):
                    nc.tensor.matmul(
                        pm[:], wsl("outWh", 0, 64, (h * 256 + mc * 128, 128)),
                        oN[(b, h)][:], start=(h == 0), stop=(h == 1))
                mt = mlp.tile([128, QS], F32R, tag=f"modT{mc}", name=f"modT{b}{mc}")
                nc.scalar.activation(mt[:], pm[:], AF.Identity,
                                     bias=rf(wsl("outb", 0, 128, (mc, 1))))
                modT.append(mt)
            # m_l = relu(h_l + modT @ mod_W + mod_b)
            mls = [[None, None], [None, None]]
            for l in range(2):
                for mc in range(2):
                    pm = p_big.tile([128, QS], F32, tag="st", name=f"pml{b}{l}{mc}")
                    for k in range(2):
                        nc.tensor.matmul(
                            pm[:], wsl("modW", 0, 128,
                                       ((l * 2 + k) * 256 + mc * 128, 128)),
                            modT[k][:], start=(k == 0), stop=(k == 1))
                    tadd = miscp.tile([128, QS], F32, tag="tadd", name="tadd")
                    nc.vector.tensor_tensor(tadd[:], pm[:], h_lT[l][mc][:], OP.add)
                    ml = mlp.tile([128, QS], F32R, tag=f"ml{l}{mc}", name=f"ml{b}{l}{mc}")
                    nc.scalar.activation(ml[:], tadd[:], AF.Relu,
                                         bias=rf(wsl("modb", 0, 128, (l * 2 + mc, 1))))
                    mls[l][mc] = ml
            # h_v1 = relu((m0 + m1) @ hv_W + hv_b)
            sum01 = []
            for mc in range(2):
                s01 = miscp.tile([128, QS], F32R, tag="s01", name="s01")
                nc.vector.tensor_tensor(s01[:], rf(mls[0][mc][:]),
                                        rf(mls[1][mc][:]), OP.add)
                sum01.append(s01)
            hv1 = []
            for mc in range(2):
                pm = p_big.tile([128, QS], F32, tag="st", name=f"phv{b}{mc}")
                for k in range(2):
                    nc.tensor.matmul(
                        pm[:], wsl("hvW", 0, 128, (k * 256 + mc * 128, 128)),
                        sum01[k][:], start=(k == 0), stop=(k == 1))
                hv = mlp.tile([128, QS], F32R, tag=f"hv{mc}", name=f"hv{b}{mc}")
                nc.scalar.activation(hv[:], pm[:], AF.Relu,
                                     bias=rf(wsl("hvb", 0, 128, (mc, 1))))
                hv1.append(hv)
            # out row = h_v0 @ outl_W[0] + h_v1 @ outl_W[1] + sum(outl_b)
            por = p_row.tile([1, QS], F32, tag="por", name=f"por{b}")
            steps = [(wsl("olW", 0, 128, (2 * k, 1)), mls[0][k]) for k in range(2)] + \
                    [(wsl("olW", 0, 128, (2 * k + 1, 1)), hv1[k]) for k in range(2)]
            for si, (lw, rv) in enumerate(steps):
                nc.tensor.matmul(por[:], lw, rv[:], start=(si == 0),
                                 stop=(si == len(steps) - 1))
            nc.scalar.activation(orows[b][:], por[:], AF.Identity,
                                 bias=rf(wsl("olb", 0, 1)))
        for b in range(B):
            nc.sync.dma_start(out_d[b : b + 1, :], orows[b][:])
        ctx.close()

    _split_multi_waits_inline(nc)
    return nc


def _split_multi_waits_inline(nc):
    """Self-contained copy of the wait-splitting post-pass."""
    for fn in nc.m.functions:
        for blk in fn.blocks:
            new_insts = []
            for inst in blk.instructions:
                si = getattr(inst, "sync_info", None)
                if si is not None and len(si.on_wait) > 1:
                    waits = list(si.on_wait)
                    for j, w in enumerate(waits[:-1]):
                        new_insts.append(mybir.InstNoOp(
                            name=f"{inst.name}-ws{j}",
                            engine=inst.engine,
                            sync_info=mybir.SyncInfo(on_wait=[w], on_update=[]),
                            bass_nofuse=True,
                        ))
                    si.on_wait = waits[-1:]
                new_insts.append(inst)
            blk.instructions = new_insts


_CACHED = {}
LAST_RESULTS = None


def kernel(**inputs):
    x = np.asarray(inputs["x"], np.float32)
    tokens = np.asarray(inputs["tokens"], np.float32)
    assert int(inputs["gD"]) == 8 and int(inputs["gH"]) == 8
    assert int(inputs["gW"]) == 8 and int(inputs["gT"]) == 8

    x0 = np.ascontiguousarray(x[0])  # (Q, 4) — reference uses x[0] for all batches
    s_all = _window_starts(x0)
    order = np.argsort(s_all, kind="stable")

    # per-core token range with sorted-query sharding
    cbases, needs = [], []
    for c in range(N_CORES):
        qidx = order[c * QS : (c + 1) * QS]
        smin, smax = int(s_all[qidx].min()), int(s_all[qidx].max())
        needs.append(smax + TOPK - smin)
        cbases.append(smin)
    nch = max(3, int(math.ceil(max(needs) / 128.0)))
    LT = nch * 128
    cbases = [max(0, min(cb, L - LT)) for cb in cbases]

    if nch not in _CACHED:
        _CACHED[nch] = build_program(nch)
    nc = _CACHED[nch]

    wpacks = _pack_weights(inputs)
    tt = tokens.transpose(0, 2, 1)   # (B, 256, 1024)

    in_maps = []
    for c in range(N_CORES):
        qidx = order[c * QS : (c + 1) * QS]
        cb = cbases[c]
        m = {"wpack0": wpacks[0], "wpack1": wpacks[1],
             "qgrid": np.ascontiguousarray(x0[qidx].T),
             "qsa": np.ascontiguousarray(
                 (s_all[qidx] - cb).astype(np.float32)[None, :])}
        for b in range(B):
            m[f"tokpack{b}"] = np.ascontiguousarray(
                np.concatenate([tt[b, 0:128, cb : cb + LT],
                                tt[b, 128:256, cb : cb + LT]], axis=1))
        in_maps.append(m)

    global LAST_RESULTS
    trace = bool(os.environ.get("KERNEL_TRACE"))
    res = run_bass_kernel_spmd(nc, in_maps, core_ids=list(range(N_CORES)),
                               trace=trace)
    LAST_RESULTS = res
    out = np.empty((B, Q), np.float32)
    for c in range(N_CORES):
        out[:, order[c * QS : (c + 1) * QS]] = res.results[c]["out"]
    return out.reshape(B, Q, 1)
